# revision 27
# baseline (speedup 1.0000x reference)
"""MultiHeadAttention forward on 8 Trainium2 NeuronCores.

Problem (hardcoded): x [4, 2048, 1024] fp32, fused QKV [1024, 3072],
out-proj [1024, 1024], 16 heads, head_dim 64, non-causal softmax.

Sharding: tensor-parallel over heads — 2 heads per core. Each core:
  1. QKV^T projection for its 2 heads (q/k/v features on partitions,
     tokens on the free axis) — bf16 matmuls, N=512. Bias+evacuation on
     DVE (tensor_scalar_add) to keep ScalarE free for exp.
  2. Flash-style attention in the S^T = K^T.T @ Q^T layout: per-head
     K=64 matmuls (PE streams 2 cols/cycle at K<=64), one wide exp per
     j-tile on ScalarE (scale=1/8 fused), O^T accumulation with a
     ones-column appended to V so row 64 is the softmax denominator.
  3. Normalize via DVE reciprocal_approx_fast + bf16 K=1 ones-matmul
     partition-broadcast, one i-chunk behind the accumulators.
  4. Per-(batch, i-chunk) AllToAll (16 small collectives) reshards
     attn_v^T from head-sharded to token-sharded 64-token slots; each
     collective is fired as soon as its chunk is normalized so comm
     fully overlaps compute and the tail only waits on a quarter-size
     A2A.
  5. Output projection per 128-token tile + bias.
Emission is finely software-pipelined: within each attention i-chunk
the j-loop is emitted S one step ahead of O (so the in-order PE queue
never head-of-line blocks on the exp), and QKV chunks of batch u+1,
out-proj tiles of batch u-1, normalizes, and collective triggers are
drained into the j-loop as filler pieces. Host reorders the
interleaved 64-token groups -> [4, 2048, 1024].
"""

import numpy as np
import ml_dtypes

import concourse.bacc as bacc
import concourse.mybir as mybir
import concourse.tile as tile
from concourse import bass_utils
from concourse.bass import ds, ts
from concourse.masks import make_identity

N_CORES = 8
P = 128
D_MODEL = 1024
N_HEAD = 16
D_HEAD = 64
BATCH = 4
SEQ = 2048
T_FULL = BATCH * SEQ          # 8192
FT = 512                      # free-dim tile (tokens) for N=512 matmuls
KO = D_MODEL // P             # 8 contraction subtiles
N_CHUNK_B = SEQ // FT         # 4 token chunks per batch
N_JT_B = SEQ // P             # 16 key tiles per batch
TBLK = SEQ // N_CORES         # 256: per-core token block within a batch
GRP = 64                      # per-(core, chunk) token group after A2A
SCALE = 1.0 / np.sqrt(D_HEAD)

f32 = mybir.dt.float32
bf16 = mybir.dt.bfloat16
AF = mybir.ActivationFunctionType

_CACHE = {}


def build():
    nc = bacc.Bacc("TRN2", target_bir_lowering=False, debug=False,
                   num_devices=N_CORES)

    xT = nc.dram_tensor("xT", (D_MODEL, T_FULL), bf16,
                        kind="ExternalInput").ap()
    wq = nc.dram_tensor("wq", (D_MODEL, P), bf16, kind="ExternalInput").ap()
    wk = nc.dram_tensor("wk", (D_MODEL, P), bf16, kind="ExternalInput").ap()
    wv = nc.dram_tensor("wv", (D_MODEL, P), bf16, kind="ExternalInput").ap()
    bqkv = nc.dram_tensor("bqkv", (3, P), f32, kind="ExternalInput").ap()
    w_out = nc.dram_tensor("w_out", (D_MODEL, D_MODEL), bf16,
                           kind="ExternalInput").ap()
    b_out = nc.dram_tensor("b_out", (P, D_MODEL), f32,
                           kind="ExternalInput").ap()
    out = nc.dram_tensor("out", (BATCH * TBLK, D_MODEL), f32,
                         kind="ExternalOutput").ap()

    with tile.TileContext(nc) as tc:
        _build_body(nc, tc, xT, wq, wk, wv, bqkv, w_out, b_out, out)

    nc.compile()
    return nc


def _build_body(nc, tc, xT, wq, wk, wv, bqkv, w_out, b_out, out):
    import contextlib
    from collections import deque
    ctx = contextlib.ExitStack()
    with ctx:
        consts = ctx.enter_context(tc.tile_pool(name="consts", bufs=1))
        xt_pool = ctx.enter_context(tc.tile_pool(name="xt", bufs=16))
        qkv_pool = ctx.enter_context(tc.tile_pool(name="qkv", bufs=2))
        v_pool = ctx.enter_context(tc.tile_pool(name="vnat", bufs=2))
        vt_pool = ctx.enter_context(tc.tile_pool(name="vt", bufs=3))
        p_pool = ctx.enter_context(tc.tile_pool(name="pexp", bufs=6))
        ev_pool = ctx.enter_context(tc.tile_pool(name="evac", bufs=6))
        posb_pool = ctx.enter_context(tc.tile_pool(name="posb", bufs=4))
        av_pool = ctx.enter_context(tc.tile_pool(name="avt", bufs=3))
        avt_pool = ctx.enter_context(tc.tile_pool(name="avtin", bufs=2))
        out_pool = ctx.enter_context(tc.tile_pool(name="outsb", bufs=3))

        ps_a = ctx.enter_context(tc.tile_pool(name="ps_a", bufs=2,
                                              space="PSUM"))
        ps_s = ctx.enter_context(tc.tile_pool(name="ps_s", bufs=2,
                                              space="PSUM"))
        ps_o = ctx.enter_context(tc.tile_pool(name="ps_o", bufs=2,
                                              space="PSUM"))

        dram = ctx.enter_context(tc.tile_pool(name="dram", bufs=1,
                                              space="DRAM"))

        # ---- early constants: only what batch-0 QKV needs ----
        wq_sb = consts.tile([P, KO, P], bf16)
        wk_sb = consts.tile([P, KO, P], bf16)
        wv_sb = consts.tile([P, KO, P], bf16)
        nc.gpsimd.dma_start(wq_sb[:],
                            wq.rearrange("(ko p) e -> p ko e", p=P))
        nc.gpsimd.dma_start(wk_sb[:],
                            wk.rearrange("(ko p) e -> p ko e", p=P))
        nc.gpsimd.dma_start(wv_sb[:],
                            wv.rearrange("(ko p) e -> p ko e", p=P))

        bqkv_sb = consts.tile([P, 3], f32)
        nc.sync.dma_start(bqkv_sb[:], bqkv.rearrange("b p -> p b"))
        bq_sb = bqkv_sb[:, 0:1]
        bk_sb = bqkv_sb[:, 1:2]
        bv_sb = bqkv_sb[:, 2:3]

        ident = consts.tile([P, P], bf16)
        make_identity(nc, ident[:])

        ones_bf = consts.tile([1, P], bf16)
        nc.vector.memset(ones_bf[:], 1.0)
        onecol_f32 = consts.tile([P, 1], f32)
        nc.vector.memset(onecol_f32[:], 1.0)

        # deferred big consts (needed only from the first out-proj on)
        wout_sb = consts.tile([P, KO, D_MODEL], bf16)
        bout_sb = consts.tile([P, D_MODEL], f32)

        # A2A buffers, one pair per (batch, half-batch):
        # [dst_core, feature(2 heads * 64), token(128, contiguous)]
        cc_in = {}
        cc_out = {}
        for u in range(BATCH):
            for hb in range(2):
                cc_in[(u, hb)] = dram.tile(
                    [N_CORES, P, 2 * GRP], bf16,
                    name=f"cc_in{u}_{hb}", tag=f"cc_in{u}_{hb}")
                cc_out[(u, hb)] = dram.tile(
                    [N_CORES, P, 2 * GRP], bf16,
                    name=f"cc_out{u}_{hb}", tag=f"cc_out{u}_{hb}")
        TG = 2 * GRP  # 128-token slot per dst core per half-batch

        # persistent per-batch tiles (2 rotating buffers each)
        qt_b = [qkv_pool.tile([P, SEQ], bf16, tag="qt", name="qt")
                for _ in range(2)]
        kt_b = [qkv_pool.tile([P, SEQ], bf16, tag="kt", name="kt")
                for _ in range(2)]
        # V natural tiles padded to 193 wide so both heads' O-matmul
        # weights can be read as full 128-column tiles (enables FWL);
        # rows 65+ of the accumulators are junk and never read.
        vn_b = [v_pool.tile([P, N_JT_B, 193], bf16, tag="v", name="v")
                for _ in range(2)]
        # ones columns + FWL padding of the V tiles are written once
        for vb in vn_b:
            for jt in range(N_JT_B):
                nc.vector.tensor_copy(vb[:, jt, 64:65], onecol_f32[:])
                nc.vector.tensor_copy(vb[:, jt, 129:130], onecol_f32[:])
                nc.vector.memset(vb[:, jt, 130:193], 0.0)

        qt = lambda u: qt_b[u % 2]
        kt = lambda u: kt_b[u % 2]
        vn = lambda u: vn_b[u % 2]

        xt_tiles = {}

        def emit_xt_dma(u, icu):
            t0 = u * SEQ + icu * FT
            xs = []
            for k in range(KO):
                x = xt_pool.tile([P, FT], bf16, tag="xt", name="xt")
                nc.sync.dma_start(x[:], xT[ts(k, P), ds(t0, FT)])
                xs.append(x)
            xt_tiles[(u, icu)] = xs

        def mm_piece(u, icu, which):
            """One projection (q/k/v) of one 512-token chunk: 8 matmuls
            + DVE bias evacuation (+ V transposes)."""
            xs = xt_tiles[(u, icu)]
            cs = ds(icu * FT, FT)
            w_sb, b_sb = ((wq_sb, bq_sb), (wk_sb, bk_sb),
                          (wv_sb, bv_sb))[which]
            ps = ps_a.tile([P, FT], f32, tag="psa", name="ps")
            for k in range(KO):
                nc.tensor.matmul(ps[:], w_sb[:, k], xs[k][:],
                                 start=(k == 0), stop=(k == KO - 1))
            if which == 0:
                nc.vector.tensor_scalar_add(qt(u)[:, cs], ps[:], b_sb[:])
            elif which == 1:
                nc.vector.tensor_scalar_add(kt(u)[:, cs], ps[:], b_sb[:])
            else:
                vt_tmp = vt_pool.tile([P, FT], bf16, tag="vt",
                                      name="vt_tmp")
                nc.vector.tensor_scalar_add(vt_tmp[:], ps[:], b_sb[:])
                for j in range(FT // P):
                    jt = icu * (FT // P) + j
                    vtr = ps_a.tile([P, P], bf16, tag="psa", name="vtr")
                    nc.tensor.transpose(vtr[:], vt_tmp[:, ts(j, P)],
                                        ident[:])
                    nc.vector.tensor_copy(vn(u)[:, jt, 0:D_HEAD],
                                          vtr[:, 0:D_HEAD])
                    nc.vector.tensor_copy(vn(u)[:, jt, 65:129],
                                          vtr[:, D_HEAD:P])

        # ---- normalize + A2A ----
        norm_state = {}

        def norm_start(u, icc, po_sb):
            """DVE half of the normalize: reciprocal of the denominators
            (row 64 of the accumulators)."""
            recbs = []
            for h in range(2):
                row = ev_pool.tile([1, FT], f32, tag="row", name="row")
                nc.vector.tensor_copy(row[:], po_sb[h][64:65, :])
                rec = ev_pool.tile([1, FT], f32, tag="rec", name="rec")
                nc.vector.reciprocal_approx_fast(rec[:], row[:])
                recb = ev_pool.tile([1, FT], bf16, tag="recb",
                                    name="recb")
                nc.vector.tensor_copy(recb[:], rec[:])
                recbs.append(recb)
            norm_state[(u, icc)] = (po_sb, recbs)

        def norm_finish(u, icc):
            """PE broadcast + DVE scale + A2A input DMA; fires the
            half-batch collective after the odd chunk."""
            po_sb, recbs = norm_state.pop((u, icc))
            cin = cc_in[(u, icc // 2)]
            g = icc % 2
            for h in range(2):
                ps_r = ps_a.tile([P, FT], f32, tag="psa", name="ps_r")
                nc.tensor.matmul(ps_r[:], ones_bf[:], recbs[h][:],
                                 start=True, stop=True)
                av = av_pool.tile([D_HEAD, FT], bf16, tag="av",
                                  name="av")
                nc.vector.tensor_tensor(av[:], po_sb[h][0:D_HEAD, :],
                                        ps_r[0:D_HEAD, :],
                                        mybir.AluOpType.mult)
                for m in range(4):
                    nc.gpsimd.dma_start(
                        cin[4 * g + m, ds(h * D_HEAD, D_HEAD), :],
                        av[:, ds(m * TG, TG)])
            if g == 1:
                nc.gpsimd.collective_compute(
                    "AllToAll", mybir.AluOpType.bypass,
                    replica_groups=[list(range(N_CORES))],
                    ins=[cin.opt()], outs=[cc_out[(u, icc // 2)].opt()])

        # ---- out-proj pieces ----
        avt_tiles = {}

        def avt_dma(u, tl):
            """Prefetch the re-sharded attn_v^T for token tile tl."""
            av_t = avt_pool.tile([P, KO, 2 * GRP], bf16, tag="avt2",
                                 name="av_t")
            nc.sync.dma_start(
                av_t[:],
                cc_out[(u, tl)][:].rearrange("s p r -> p s r"))
            avt_tiles[(u, tl)] = av_t

        def proj_piece(u, tl, fc):
            av_t = avt_tiles[(u, tl)]
            ps_d = ps_a.tile([P, FT], f32, tag="psa", name="ps_d")
            for k in range(KO):
                nc.tensor.matmul(ps_d[:], av_t[:, k],
                                 wout_sb[:, k, ds(fc * FT, FT)],
                                 start=(k == 0), stop=(k == KO - 1))
            o_sb = out_pool.tile([P, FT], f32, tag="osb",
                                 name="o_sb")
            nc.vector.tensor_tensor(o_sb[:], ps_d[:],
                                    bout_sb[:, ds(fc * FT, FT)],
                                    mybir.AluOpType.add)
            nc.gpsimd.dma_start(
                out[ds(u * TBLK + tl * P, P), ds(fc * FT, FT)],
                o_sb[:])

        # ---- attention i-chunk with interleaved fillers ----
        def attention_slot(u, icc, fillers):
            fq = deque(fillers)
            qs = ds(icc * FT, FT)
            po = [ps_o.tile([P, FT], f32, tag="pso", name="pso")
                  for _ in range(2)]

            def emit_s(j):
                ps_sc = ps_s.tile([P, 2, FT], f32, tag="pss",
                                  name="ps_sc")
                for h in range(2):
                    hp = ds(h * D_HEAD, D_HEAD)
                    nc.tensor.matmul(ps_sc[:, h, :],
                                     kt(u)[hp, ts(j, P)],
                                     qt(u)[hp, qs],
                                     start=True, stop=True)
                return ps_sc

            ps_sc = emit_s(0)
            for j in range(N_JT_B):
                pexp = p_pool.tile([P, 2, FT], bf16, tag="pexp",
                                   name="pexp")
                nc.scalar.activation(pexp[:], ps_sc[:], AF.Exp,
                                     scale=float(SCALE))
                if j + 1 < N_JT_B:
                    ps_sc = emit_s(j + 1)
                st, sp = (j == 0), (j == N_JT_B - 1)
                for h in range(2):
                    nc.tensor.matmul(po[h][:],
                                     vn(u)[:, j, ds(h * 65, P)],
                                     pexp[:, h, :], start=st, stop=sp)
                if j >= 2 and fq:
                    fq.popleft()()
            while fq:
                fq.popleft()()
            po_sb = [None, None]
            for h in range(2):
                po_sb[h] = posb_pool.tile([P, FT], f32, tag="posb",
                                          name="po_sb")
                nc.vector.tensor_copy(po_sb[h][0:65, :], po[h][0:65, :])
            return po_sb

        # ---- prologue: batch-0 QKV, deferred big consts ----
        emit_xt_dma(0, 0)
        emit_xt_dma(0, 1)
        nc.gpsimd.dma_start(wout_sb[:],
                            w_out.rearrange("(ko p) f -> p ko f", p=P))
        nc.gpsimd.dma_start(bout_sb[:], b_out[:])
        for which in range(3):
            mm_piece(0, 0, which)
        emit_xt_dma(0, 2)
        for which in range(3):
            mm_piece(0, 1, which)
        emit_xt_dma(0, 3)
        for which in range(3):
            mm_piece(0, 2, which)
        for which in range(3):
            mm_piece(0, 3, which)

        # ---- main software-pipelined slot loop ----
        prev = None          # (u, icc) whose normalize is pending
        po_prev = None
        for u in range(BATCH):
            for icc in range(N_CHUNK_B):
                if u + 1 < BATCH:
                    emit_xt_dma(u + 1, icc)
                if prev is not None:
                    norm_start(*prev, po_prev)
                fillers = []
                if prev is not None:
                    pu, pic = prev
                    fillers.append(lambda pu=pu, pic=pic:
                                   norm_finish(pu, pic))
                if u + 1 < BATCH:
                    for which in range(3):
                        fillers.append(
                            lambda w=which, uu=u + 1, ii=icc:
                            mm_piece(uu, ii, w))
                # out-proj / avt prefetch schedule
                if icc == 0 and u >= 1:
                    fillers.append(lambda uu=u - 1: proj_piece(uu, 0, 0))
                    fillers.append(lambda uu=u - 1: proj_piece(uu, 0, 1))
                if icc == 1 and u >= 1:
                    fillers.append(lambda uu=u - 1: avt_dma(uu, 1))
                if icc == 2 and u >= 1:
                    fillers.append(lambda uu=u - 1: proj_piece(uu, 1, 0))
                    fillers.append(lambda uu=u - 1: proj_piece(uu, 1, 1))
                if icc == 3:
                    fillers.append(lambda uu=u: avt_dma(uu, 0))

                po_sb = attention_slot(u, icc, fillers)
                prev = (u, icc)
                po_prev = po_sb

        # ---- tail ----
        norm_start(*prev, po_prev)
        norm_finish(*prev)
        proj_piece(BATCH - 1, 0, 0)
        proj_piece(BATCH - 1, 0, 1)
        avt_dma(BATCH - 1, 1)
        proj_piece(BATCH - 1, 1, 0)
        proj_piece(BATCH - 1, 1, 1)


def _prep_inputs(x, w_qkv, b_qkv, w_out, b_out):
    x = np.asarray(x, dtype=np.float32)
    w_qkv = np.asarray(w_qkv, dtype=np.float32)
    b_qkv = np.asarray(b_qkv, dtype=np.float32)
    w_out = np.asarray(w_out, dtype=np.float32)
    b_out = np.asarray(b_out, dtype=np.float32)
    bf = ml_dtypes.bfloat16

    xT = np.ascontiguousarray(x.reshape(T_FULL, D_MODEL).T).astype(bf)
    w_out_bf = w_out.astype(bf)
    bout_bc = np.ascontiguousarray(np.broadcast_to(b_out, (P, D_MODEL)))

    in_maps = []
    for c in range(N_CORES):
        cols = slice(c * P, (c + 1) * P)
        in_maps.append({
            "xT": xT,
            "wq": np.ascontiguousarray(w_qkv[:, 0:D_MODEL][:, cols]).astype(bf),
            "wk": np.ascontiguousarray(
                w_qkv[:, D_MODEL:2 * D_MODEL][:, cols]).astype(bf),
            "wv": np.ascontiguousarray(
                w_qkv[:, 2 * D_MODEL:3 * D_MODEL][:, cols]).astype(bf),
            "bqkv": np.ascontiguousarray(np.stack([
                b_qkv[0:D_MODEL][cols],
                b_qkv[D_MODEL:2 * D_MODEL][cols],
                b_qkv[2 * D_MODEL:3 * D_MODEL][cols]])),
            "w_out": w_out_bf,
            "b_out": bout_bc,
        })
    return in_maps


def kernel(x, w_qkv, b_qkv, w_out, b_out, _trace=False):
    if "nc" not in _CACHE:
        _CACHE["nc"] = build()
    nc = _CACHE["nc"]
    in_maps = _prep_inputs(x, w_qkv, b_qkv, w_out, b_out)
    res = bass_utils.run_bass_kernel_spmd(
        nc, in_maps, core_ids=list(range(N_CORES)), trace=_trace)
    _CACHE["last_result"] = res
    # reorder interleaved 128-token groups:
    # core c's out row (u*256 + hb*128 + r) is global token
    # u*2048 + hb*1024 + c*128 + r
    full = np.empty((T_FULL, D_MODEL), np.float32)
    for c in range(N_CORES):
        r = res.results[c]["out"]
        for u in range(BATCH):
            for hb in range(2):
                src = u * TBLK + hb * 128
                dst = u * SEQ + hb * 1024 + c * 128
                full[dst:dst + 128] = r[src:src + 128]
    return full.reshape(BATCH, SEQ, D_MODEL)


if __name__ == "__main__":
    rng = np.random.default_rng(0)
    x = rng.standard_normal((BATCH, SEQ, D_MODEL), dtype=np.float32)
    w_qkv = rng.standard_normal((D_MODEL, 3 * D_MODEL),
                                dtype=np.float32) / 32.0
    b_qkv = rng.standard_normal((3 * D_MODEL,), dtype=np.float32) * 0.02
    w_out = rng.standard_normal((D_MODEL, D_MODEL), dtype=np.float32) / 32.0
    b_out = rng.standard_normal((D_MODEL,), dtype=np.float32) * 0.02
    got = kernel(x, w_qkv, b_qkv, w_out, b_out)
    print("out shape:", got.shape)


# revision 28
# speedup vs baseline: 1.2007x; 1.2007x over previous
"""MultiHeadAttention forward on 8 Trainium2 NeuronCores.

Problem (hardcoded): x [4, 2048, 1024] fp32, fused QKV [1024, 3072],
out-proj [1024, 1024], 16 heads, head_dim 64, non-causal softmax.

Sharding: tensor-parallel over heads — 2 heads per core. Each core:
  1. QKV^T projection for its 2 heads (q/k/v features on partitions,
     tokens on the free axis) — bf16 matmuls, N=512. Bias+evacuation on
     DVE (tensor_scalar_add) to keep ScalarE free for exp.
  2. Flash-style attention in the S^T = K^T.T @ Q^T layout: per-head
     K=64 matmuls (PE streams 2 cols/cycle at K<=64), one wide exp per
     j-tile on ScalarE (scale=1/8 fused), O^T accumulation with a
     ones-column appended to V so row 64 is the softmax denominator.
  3. Normalize via DVE reciprocal_approx_fast + bf16 K=1 ones-matmul
     partition-broadcast, one i-chunk behind the accumulators.
  4. Per-(batch, i-chunk) AllToAll (16 small collectives) reshards
     attn_v^T from head-sharded to token-sharded 64-token slots; each
     collective is fired as soon as its chunk is normalized so comm
     fully overlaps compute and the tail only waits on a quarter-size
     A2A.
  5. Output projection per 128-token tile + bias.
Emission is finely software-pipelined: within each attention i-chunk
the j-loop is emitted S one step ahead of O (so the in-order PE queue
never head-of-line blocks on the exp), and QKV chunks of batch u+1,
out-proj tiles of batch u-1, normalizes, and collective triggers are
drained into the j-loop as filler pieces. Host reorders the
interleaved 64-token groups -> [4, 2048, 1024].
"""

import numpy as np
import ml_dtypes

import concourse.bacc as bacc
import concourse.mybir as mybir
import concourse.tile as tile
from concourse import bass_utils
from concourse.bass import ds, ts
from concourse.masks import make_identity

N_CORES = 8
P = 128
D_MODEL = 1024
N_HEAD = 16
D_HEAD = 64
BATCH = 4
SEQ = 2048
T_FULL = BATCH * SEQ          # 8192
FT = 512                      # free-dim tile (tokens) for N=512 matmuls
KO = D_MODEL // P             # 8 contraction subtiles
N_CHUNK_B = SEQ // FT         # 4 token chunks per batch
N_JT_B = SEQ // P             # 16 key tiles per batch
TBLK = SEQ // N_CORES         # 256: per-core token block within a batch
GRP = 64                      # per-(core, chunk) token group after A2A
SCALE = 1.0 / np.sqrt(D_HEAD)

f32 = mybir.dt.float32
bf16 = mybir.dt.bfloat16
AF = mybir.ActivationFunctionType

_CACHE = {}


def build():
    nc = bacc.Bacc("TRN2", target_bir_lowering=False, debug=False,
                   num_devices=N_CORES)

    xT = nc.dram_tensor("xT", (D_MODEL, T_FULL), bf16,
                        kind="ExternalInput").ap()
    wq = nc.dram_tensor("wq", (D_MODEL, P), bf16, kind="ExternalInput").ap()
    wk = nc.dram_tensor("wk", (D_MODEL, P), bf16, kind="ExternalInput").ap()
    wv = nc.dram_tensor("wv", (D_MODEL, P), bf16, kind="ExternalInput").ap()
    bqkv = nc.dram_tensor("bqkv", (3, P), f32, kind="ExternalInput").ap()
    w_out = nc.dram_tensor("w_out", (D_MODEL, D_MODEL), bf16,
                           kind="ExternalInput").ap()
    b_out = nc.dram_tensor("b_out", (P, D_MODEL), f32,
                           kind="ExternalInput").ap()
    out = nc.dram_tensor("out", (BATCH * TBLK, D_MODEL), f32,
                         kind="ExternalOutput").ap()

    with tile.TileContext(nc) as tc:
        _build_body(nc, tc, xT, wq, wk, wv, bqkv, w_out, b_out, out)

    nc.compile()
    return nc


def _build_body(nc, tc, xT, wq, wk, wv, bqkv, w_out, b_out, out):
    import contextlib
    from collections import deque
    ctx = contextlib.ExitStack()
    with ctx:
        consts = ctx.enter_context(tc.tile_pool(name="consts", bufs=1))
        xt_pool = ctx.enter_context(tc.tile_pool(name="xt", bufs=16))
        qkv_pool = ctx.enter_context(tc.tile_pool(name="qkv", bufs=2))
        v_pool = ctx.enter_context(tc.tile_pool(name="vnat", bufs=2))
        vt_pool = ctx.enter_context(tc.tile_pool(name="vt", bufs=3))
        p_pool = ctx.enter_context(tc.tile_pool(name="pexp", bufs=6))
        ev_pool = ctx.enter_context(tc.tile_pool(name="evac", bufs=6))
        posb_pool = ctx.enter_context(tc.tile_pool(name="posb", bufs=4))
        av_pool = ctx.enter_context(tc.tile_pool(name="avt", bufs=3))
        avt_pool = ctx.enter_context(tc.tile_pool(name="avtin", bufs=2))
        out_pool = ctx.enter_context(tc.tile_pool(name="outsb", bufs=3))

        ps_a = ctx.enter_context(tc.tile_pool(name="ps_a", bufs=2,
                                              space="PSUM"))
        ps_s = ctx.enter_context(tc.tile_pool(name="ps_s", bufs=2,
                                              space="PSUM"))
        ps_o = ctx.enter_context(tc.tile_pool(name="ps_o", bufs=2,
                                              space="PSUM"))

        dram = ctx.enter_context(tc.tile_pool(name="dram", bufs=1,
                                              space="DRAM"))

        # ---- early constants: only what batch-0 QKV needs ----
        wq_sb = consts.tile([P, KO, P], bf16)
        wk_sb = consts.tile([P, KO, P], bf16)
        wv_sb = consts.tile([P, KO, P], bf16)
        nc.gpsimd.dma_start(wq_sb[:],
                            wq.rearrange("(ko p) e -> p ko e", p=P))
        nc.gpsimd.dma_start(wk_sb[:],
                            wk.rearrange("(ko p) e -> p ko e", p=P))
        nc.gpsimd.dma_start(wv_sb[:],
                            wv.rearrange("(ko p) e -> p ko e", p=P))

        bqkv_sb = consts.tile([P, 3], f32)
        nc.sync.dma_start(bqkv_sb[:], bqkv.rearrange("b p -> p b"))
        bq_sb = bqkv_sb[:, 0:1]
        bk_sb = bqkv_sb[:, 1:2]
        bv_sb = bqkv_sb[:, 2:3]

        ident = consts.tile([P, P], bf16)
        make_identity(nc, ident[:])

        ones_bf = consts.tile([1, P], bf16)
        nc.vector.memset(ones_bf[:], 1.0)
        onecol_f32 = consts.tile([P, 1], f32)
        nc.vector.memset(onecol_f32[:], 1.0)

        # deferred big consts (needed only from the first out-proj on)
        wout_sb = consts.tile([P, KO, D_MODEL], bf16)
        bout_sb = consts.tile([P, D_MODEL], f32)

        # A2A buffers, one pair per (batch, half-batch):
        # [dst_core, feature(2 heads * 64), token(128, contiguous)]
        cc_in = {}
        cc_out = {}
        for u in range(BATCH):
            for hb in range(2):
                cc_in[(u, hb)] = dram.tile(
                    [N_CORES, P, 2 * GRP], bf16,
                    name=f"cc_in{u}_{hb}", tag=f"cc_in{u}_{hb}")
                cc_out[(u, hb)] = dram.tile(
                    [N_CORES, P, 2 * GRP], bf16,
                    name=f"cc_out{u}_{hb}", tag=f"cc_out{u}_{hb}")
        TG = 2 * GRP  # 128-token slot per dst core per half-batch

        # persistent per-batch tiles (2 rotating buffers each)
        qt_b = [qkv_pool.tile([P, SEQ], bf16, tag="qt", name="qt")
                for _ in range(2)]
        kt_b = [qkv_pool.tile([P, SEQ], bf16, tag="kt", name="kt")
                for _ in range(2)]
        # V natural tiles padded to 193 wide so both heads' O-matmul
        # weights can be read as full 128-column tiles (enables FWL);
        # rows 65+ of the accumulators are junk and never read.
        vn_b = [v_pool.tile([P, N_JT_B, 193], bf16, tag="v", name="v")
                for _ in range(2)]
        # ones columns + FWL padding of the V tiles are written once
        for vb in vn_b:
            for jt in range(N_JT_B):
                nc.vector.tensor_copy(vb[:, jt, 64:65], onecol_f32[:])
                nc.vector.tensor_copy(vb[:, jt, 129:130], onecol_f32[:])
                nc.vector.memset(vb[:, jt, 130:193], 0.0)

        qt = lambda u: qt_b[u % 2]
        kt = lambda u: kt_b[u % 2]
        vn = lambda u: vn_b[u % 2]

        xt_tiles = {}

        def emit_xt_dma(u, icu):
            t0 = u * SEQ + icu * FT
            xs = []
            for k in range(KO):
                x = xt_pool.tile([P, FT], bf16, tag="xt", name="xt")
                nc.sync.dma_start(x[:], xT[ts(k, P), ds(t0, FT)])
                xs.append(x)
            xt_tiles[(u, icu)] = xs

        def mm_piece(u, icu, which):
            """One projection (q/k/v) of one 512-token chunk: 8 matmuls
            + DVE bias evacuation (+ V transposes)."""
            xs = xt_tiles[(u, icu)]
            cs = ds(icu * FT, FT)
            w_sb, b_sb = ((wq_sb, bq_sb), (wk_sb, bk_sb),
                          (wv_sb, bv_sb))[which]
            ps = ps_a.tile([P, FT], f32, tag="psa", name="ps")
            for k in range(KO):
                nc.tensor.matmul(ps[:], w_sb[:, k], xs[k][:],
                                 start=(k == 0), stop=(k == KO - 1))
            if which == 0:
                nc.vector.tensor_scalar_add(qt(u)[:, cs], ps[:], b_sb[:])
            elif which == 1:
                nc.vector.tensor_scalar_add(kt(u)[:, cs], ps[:], b_sb[:])
            else:
                vt_tmp = vt_pool.tile([P, FT], bf16, tag="vt",
                                      name="vt_tmp")
                nc.vector.tensor_scalar_add(vt_tmp[:], ps[:], b_sb[:])
                for j in range(FT // P):
                    jt = icu * (FT // P) + j
                    vtr = ps_a.tile([P, P], bf16, tag="psa", name="vtr")
                    nc.tensor.transpose(vtr[:], vt_tmp[:, ts(j, P)],
                                        ident[:])
                    nc.vector.tensor_copy(vn(u)[:, jt, 0:D_HEAD],
                                          vtr[:, 0:D_HEAD])
                    nc.vector.tensor_copy(vn(u)[:, jt, 65:129],
                                          vtr[:, D_HEAD:P])

        # ---- normalize + A2A ----
        norm_state = {}

        def norm_start(u, icc, po_sb):
            """DVE half of the normalize: reciprocal of the denominators
            (row 64 of the accumulators)."""
            recbs = []
            for h in range(2):
                row = ev_pool.tile([1, FT], f32, tag="row", name="row")
                nc.vector.tensor_copy(row[:], po_sb[h][64:65, :])
                rec = ev_pool.tile([1, FT], f32, tag="rec", name="rec")
                nc.vector.reciprocal_approx_fast(rec[:], row[:])
                recb = ev_pool.tile([1, FT], bf16, tag="recb",
                                    name="recb")
                nc.vector.tensor_copy(recb[:], rec[:])
                recbs.append(recb)
            norm_state[(u, icc)] = (po_sb, recbs)

        def norm_finish(u, icc):
            """PE broadcast + DVE scale + A2A input DMA; fires the
            half-batch collective after the odd chunk."""
            po_sb, recbs = norm_state.pop((u, icc))
            cin = cc_in[(u, icc // 2)]
            g = icc % 2
            for h in range(2):
                ps_r = ps_a.tile([P, FT], f32, tag="psa", name="ps_r")
                nc.tensor.matmul(ps_r[:], ones_bf[:], recbs[h][:],
                                 start=True, stop=True)
                av = av_pool.tile([D_HEAD, FT], bf16, tag="av",
                                  name="av")
                nc.vector.tensor_tensor(av[:], po_sb[h][0:D_HEAD, :],
                                        ps_r[0:D_HEAD, :],
                                        mybir.AluOpType.mult)
                for m in range(4):
                    nc.gpsimd.dma_start(
                        cin[4 * g + m, ds(h * D_HEAD, D_HEAD), :],
                        av[:, ds(m * TG, TG)])
            if g == 1:
                nc.gpsimd.collective_compute(
                    "AllToAll", mybir.AluOpType.bypass,
                    replica_groups=[list(range(N_CORES))],
                    ins=[cin.opt()], outs=[cc_out[(u, icc // 2)].opt()])

        # ---- out-proj pieces ----
        avt_tiles = {}

        def avt_dma(u, tl):
            """Prefetch the re-sharded attn_v^T for token tile tl."""
            av_t = avt_pool.tile([P, KO, 2 * GRP], bf16, tag="avt2",
                                 name="av_t")
            nc.sync.dma_start(
                av_t[:],
                cc_out[(u, tl)][:].rearrange("s p r -> p s r"))
            avt_tiles[(u, tl)] = av_t

        def proj_piece(u, tl, fc):
            av_t = avt_tiles[(u, tl)]
            ps_d = ps_a.tile([P, FT], f32, tag="psa", name="ps_d")
            for k in range(KO):
                nc.tensor.matmul(ps_d[:], av_t[:, k],
                                 wout_sb[:, k, ds(fc * FT, FT)],
                                 start=(k == 0), stop=(k == KO - 1))
            o_sb = out_pool.tile([P, FT], f32, tag="osb",
                                 name="o_sb")
            nc.vector.tensor_tensor(o_sb[:], ps_d[:],
                                    bout_sb[:, ds(fc * FT, FT)],
                                    mybir.AluOpType.add)
            nc.gpsimd.dma_start(
                out[ds(u * TBLK + tl * P, P), ds(fc * FT, FT)],
                o_sb[:])

        # ---- attention i-chunk with interleaved fillers ----
        def attention_slot(u, icc, fillers):
            fq = deque(fillers)
            qs = ds(icc * FT, FT)
            po = [ps_o.tile([P, FT], f32, tag="pso", name="pso")
                  for _ in range(2)]

            def emit_s(j):
                ps_sc = ps_s.tile([P, 2, FT], f32, tag="pss",
                                  name="ps_sc")
                for h in range(2):
                    hp = ds(h * D_HEAD, D_HEAD)
                    nc.tensor.matmul(ps_sc[:, h, :],
                                     kt(u)[hp, ts(j, P)],
                                     qt(u)[hp, qs],
                                     start=True, stop=True)
                return ps_sc

            for j in range(N_JT_B):
                ps_sc = emit_s(j)
                pexp = p_pool.tile([P, 2, FT], bf16, tag="pexp",
                                   name="pexp")
                nc.scalar.activation(pexp[:], ps_sc[:], AF.Exp,
                                     scale=float(SCALE))
                st, sp = (j == 0), (j == N_JT_B - 1)
                for h in range(2):
                    nc.tensor.matmul(po[h][:],
                                     vn(u)[:, j, ds(h * 65, P)],
                                     pexp[:, h, :], start=st, stop=sp)
            while fq:
                fq.popleft()()
            po_sb = [None, None]
            for h in range(2):
                po_sb[h] = posb_pool.tile([P, FT], f32, tag="posb",
                                          name="po_sb")
                nc.vector.tensor_copy(po_sb[h][0:65, :], po[h][0:65, :])
            return po_sb

        # ---- prologue: batch-0 QKV, deferred big consts ----
        emit_xt_dma(0, 0)
        emit_xt_dma(0, 1)
        nc.gpsimd.dma_start(wout_sb[:],
                            w_out.rearrange("(ko p) f -> p ko f", p=P))
        nc.gpsimd.dma_start(bout_sb[:], b_out[:])
        for which in range(3):
            mm_piece(0, 0, which)
        emit_xt_dma(0, 2)
        for which in range(3):
            mm_piece(0, 1, which)
        emit_xt_dma(0, 3)
        for which in range(3):
            mm_piece(0, 2, which)
        for which in range(3):
            mm_piece(0, 3, which)

        # ---- main software-pipelined slot loop ----
        prev = None          # (u, icc) whose normalize is pending
        po_prev = None
        for u in range(BATCH):
            for icc in range(N_CHUNK_B):
                if u + 1 < BATCH:
                    emit_xt_dma(u + 1, icc)
                if prev is not None:
                    norm_start(*prev, po_prev)
                fillers = []
                if prev is not None:
                    pu, pic = prev
                    fillers.append(lambda pu=pu, pic=pic:
                                   norm_finish(pu, pic))
                if u + 1 < BATCH:
                    for which in range(3):
                        fillers.append(
                            lambda w=which, uu=u + 1, ii=icc:
                            mm_piece(uu, ii, w))
                # out-proj / avt prefetch schedule
                if icc == 0 and u >= 1:
                    fillers.append(lambda uu=u - 1: proj_piece(uu, 0, 0))
                    fillers.append(lambda uu=u - 1: proj_piece(uu, 0, 1))
                if icc == 1 and u >= 1:
                    fillers.append(lambda uu=u - 1: avt_dma(uu, 1))
                if icc == 2 and u >= 1:
                    fillers.append(lambda uu=u - 1: proj_piece(uu, 1, 0))
                    fillers.append(lambda uu=u - 1: proj_piece(uu, 1, 1))
                if icc == 3:
                    fillers.append(lambda uu=u: avt_dma(uu, 0))

                po_sb = attention_slot(u, icc, fillers)
                prev = (u, icc)
                po_prev = po_sb

        # ---- tail ----
        norm_start(*prev, po_prev)
        norm_finish(*prev)
        proj_piece(BATCH - 1, 0, 0)
        proj_piece(BATCH - 1, 0, 1)
        avt_dma(BATCH - 1, 1)
        proj_piece(BATCH - 1, 1, 0)
        proj_piece(BATCH - 1, 1, 1)


def _prep_inputs(x, w_qkv, b_qkv, w_out, b_out):
    x = np.asarray(x, dtype=np.float32)
    w_qkv = np.asarray(w_qkv, dtype=np.float32)
    b_qkv = np.asarray(b_qkv, dtype=np.float32)
    w_out = np.asarray(w_out, dtype=np.float32)
    b_out = np.asarray(b_out, dtype=np.float32)
    bf = ml_dtypes.bfloat16

    xT = np.ascontiguousarray(x.reshape(T_FULL, D_MODEL).T).astype(bf)
    w_out_bf = w_out.astype(bf)
    bout_bc = np.ascontiguousarray(np.broadcast_to(b_out, (P, D_MODEL)))

    in_maps = []
    for c in range(N_CORES):
        cols = slice(c * P, (c + 1) * P)
        in_maps.append({
            "xT": xT,
            "wq": np.ascontiguousarray(w_qkv[:, 0:D_MODEL][:, cols]).astype(bf),
            "wk": np.ascontiguousarray(
                w_qkv[:, D_MODEL:2 * D_MODEL][:, cols]).astype(bf),
            "wv": np.ascontiguousarray(
                w_qkv[:, 2 * D_MODEL:3 * D_MODEL][:, cols]).astype(bf),
            "bqkv": np.ascontiguousarray(np.stack([
                b_qkv[0:D_MODEL][cols],
                b_qkv[D_MODEL:2 * D_MODEL][cols],
                b_qkv[2 * D_MODEL:3 * D_MODEL][cols]])),
            "w_out": w_out_bf,
            "b_out": bout_bc,
        })
    return in_maps


def kernel(x, w_qkv, b_qkv, w_out, b_out, _trace=False):
    if "nc" not in _CACHE:
        _CACHE["nc"] = build()
    nc = _CACHE["nc"]
    in_maps = _prep_inputs(x, w_qkv, b_qkv, w_out, b_out)
    res = bass_utils.run_bass_kernel_spmd(
        nc, in_maps, core_ids=list(range(N_CORES)), trace=_trace)
    _CACHE["last_result"] = res
    # reorder interleaved 128-token groups:
    # core c's out row (u*256 + hb*128 + r) is global token
    # u*2048 + hb*1024 + c*128 + r
    full = np.empty((T_FULL, D_MODEL), np.float32)
    for c in range(N_CORES):
        r = res.results[c]["out"]
        for u in range(BATCH):
            for hb in range(2):
                src = u * TBLK + hb * 128
                dst = u * SEQ + hb * 1024 + c * 128
                full[dst:dst + 128] = r[src:src + 128]
    return full.reshape(BATCH, SEQ, D_MODEL)


if __name__ == "__main__":
    rng = np.random.default_rng(0)
    x = rng.standard_normal((BATCH, SEQ, D_MODEL), dtype=np.float32)
    w_qkv = rng.standard_normal((D_MODEL, 3 * D_MODEL),
                                dtype=np.float32) / 32.0
    b_qkv = rng.standard_normal((3 * D_MODEL,), dtype=np.float32) * 0.02
    w_out = rng.standard_normal((D_MODEL, D_MODEL), dtype=np.float32) / 32.0
    b_out = rng.standard_normal((D_MODEL,), dtype=np.float32) * 0.02
    got = kernel(x, w_qkv, b_qkv, w_out, b_out)
    print("out shape:", got.shape)


# revision 30
# speedup vs baseline: 1.2367x; 1.0299x over previous
"""MultiHeadAttention forward on 8 Trainium2 NeuronCores.

Problem (hardcoded): x [4, 2048, 1024] fp32, fused QKV [1024, 3072],
out-proj [1024, 1024], 16 heads, head_dim 64, non-causal softmax.

Sharding: tensor-parallel over heads — 2 heads per core. Each core:
  1. QKV^T projection for its 2 heads (q/k/v features on partitions,
     tokens on the free axis) — bf16 matmuls, N=512. Bias+evacuation on
     DVE (tensor_scalar_add) to keep ScalarE free for exp.
  2. Flash-style attention in the S^T = K^T.T @ Q^T layout: per-head
     K=64 matmuls (PE streams 2 cols/cycle at K<=64), one wide exp per
     j-tile on ScalarE (scale=1/8 fused), O^T accumulation with a
     ones-column appended to V so row 64 is the softmax denominator.
  3. Normalize via DVE reciprocal_approx_fast + bf16 K=1 ones-matmul
     partition-broadcast, one i-chunk behind the accumulators.
  4. Per-(batch, i-chunk) AllToAll (16 small collectives) reshards
     attn_v^T from head-sharded to token-sharded 64-token slots; each
     collective is fired as soon as its chunk is normalized so comm
     fully overlaps compute and the tail only waits on a quarter-size
     A2A.
  5. Output projection per 128-token tile + bias.
Emission is finely software-pipelined: within each attention i-chunk
the j-loop is emitted S one step ahead of O (so the in-order PE queue
never head-of-line blocks on the exp), and QKV chunks of batch u+1,
out-proj tiles of batch u-1, normalizes, and collective triggers are
drained into the j-loop as filler pieces. Host reorders the
interleaved 64-token groups -> [4, 2048, 1024].
"""

import numpy as np
import ml_dtypes

import concourse.bacc as bacc
import concourse.mybir as mybir
import concourse.tile as tile
from concourse import bass_utils
from concourse.bass import ds, ts
from concourse.masks import make_identity

N_CORES = 8
P = 128
D_MODEL = 1024
N_HEAD = 16
D_HEAD = 64
BATCH = 4
SEQ = 2048
T_FULL = BATCH * SEQ          # 8192
FT = 512                      # free-dim tile (tokens) for N=512 matmuls
KO = D_MODEL // P             # 8 contraction subtiles
N_CHUNK_B = SEQ // FT         # 4 token chunks per batch
N_JT_B = SEQ // P             # 16 key tiles per batch
TBLK = SEQ // N_CORES         # 256: per-core token block within a batch
GRP = 64                      # per-(core, chunk) token group after A2A
SCALE = 1.0 / np.sqrt(D_HEAD)

f32 = mybir.dt.float32
bf16 = mybir.dt.bfloat16
AF = mybir.ActivationFunctionType

_CACHE = {}


def build():
    nc = bacc.Bacc("TRN2", target_bir_lowering=False, debug=False,
                   num_devices=N_CORES)

    xT = nc.dram_tensor("xT", (D_MODEL, T_FULL), bf16,
                        kind="ExternalInput").ap()
    wq = nc.dram_tensor("wq", (D_MODEL, P), bf16, kind="ExternalInput").ap()
    wk = nc.dram_tensor("wk", (D_MODEL, P), bf16, kind="ExternalInput").ap()
    wv = nc.dram_tensor("wv", (D_MODEL, P), bf16, kind="ExternalInput").ap()
    bqkv = nc.dram_tensor("bqkv", (3, P), f32, kind="ExternalInput").ap()
    w_out = nc.dram_tensor("w_out", (D_MODEL, D_MODEL), bf16,
                           kind="ExternalInput").ap()
    b_out = nc.dram_tensor("b_out", (P, D_MODEL), f32,
                           kind="ExternalInput").ap()
    out = nc.dram_tensor("out", (BATCH * TBLK, D_MODEL), f32,
                         kind="ExternalOutput").ap()

    with tile.TileContext(nc) as tc:
        _build_body(nc, tc, xT, wq, wk, wv, bqkv, w_out, b_out, out)

    nc.compile()
    return nc


def _build_body(nc, tc, xT, wq, wk, wv, bqkv, w_out, b_out, out):
    import contextlib
    from collections import deque
    ctx = contextlib.ExitStack()
    with ctx:
        consts = ctx.enter_context(tc.tile_pool(name="consts", bufs=1))
        xt_pool = ctx.enter_context(tc.tile_pool(name="xt", bufs=16))
        qkv_pool = ctx.enter_context(tc.tile_pool(name="qkv", bufs=2))
        v_pool = ctx.enter_context(tc.tile_pool(name="vnat", bufs=2))
        vt_pool = ctx.enter_context(tc.tile_pool(name="vt", bufs=3))
        p_pool = ctx.enter_context(tc.tile_pool(name="pexp", bufs=6))
        ev_pool = ctx.enter_context(tc.tile_pool(name="evac", bufs=6))
        posb_pool = ctx.enter_context(tc.tile_pool(name="posb", bufs=4))
        av_pool = ctx.enter_context(tc.tile_pool(name="avt", bufs=3))
        avt_pool = ctx.enter_context(tc.tile_pool(name="avtin", bufs=2))
        out_pool = ctx.enter_context(tc.tile_pool(name="outsb", bufs=3))

        ps_a = ctx.enter_context(tc.tile_pool(name="ps_a", bufs=2,
                                              space="PSUM"))
        ps_s = ctx.enter_context(tc.tile_pool(name="ps_s", bufs=2,
                                              space="PSUM"))
        ps_o = ctx.enter_context(tc.tile_pool(name="ps_o", bufs=2,
                                              space="PSUM"))

        dram = ctx.enter_context(tc.tile_pool(name="dram", bufs=1,
                                              space="DRAM"))

        # ---- early constants: only what batch-0 QKV needs ----
        wq_sb = consts.tile([P, KO, P], bf16)
        wk_sb = consts.tile([P, KO, P], bf16)
        wv_sb = consts.tile([P, KO, P], bf16)
        nc.gpsimd.dma_start(wq_sb[:],
                            wq.rearrange("(ko p) e -> p ko e", p=P))
        nc.gpsimd.dma_start(wk_sb[:],
                            wk.rearrange("(ko p) e -> p ko e", p=P))
        nc.gpsimd.dma_start(wv_sb[:],
                            wv.rearrange("(ko p) e -> p ko e", p=P))

        bqkv_sb = consts.tile([P, 3], f32)
        nc.sync.dma_start(bqkv_sb[:], bqkv.rearrange("b p -> p b"))
        bq_sb = bqkv_sb[:, 0:1]
        bk_sb = bqkv_sb[:, 1:2]
        bv_sb = bqkv_sb[:, 2:3]

        ident = consts.tile([P, P], bf16)
        make_identity(nc, ident[:])

        ones_bf = consts.tile([1, P], bf16)
        nc.vector.memset(ones_bf[:], 1.0)
        onecol_f32 = consts.tile([P, 1], f32)
        nc.vector.memset(onecol_f32[:], 1.0)

        # deferred big consts (needed only from the first out-proj on)
        wout_sb = consts.tile([P, KO, D_MODEL], bf16)
        bout_sb = consts.tile([P, D_MODEL], f32)

        # A2A buffers, one pair per (batch, half-batch):
        # [dst_core, feature(2 heads * 64), token(128, contiguous)]
        cc_in = {}
        cc_out = {}
        for u in range(BATCH):
            for hb in range(2):
                cc_in[(u, hb)] = dram.tile(
                    [N_CORES, P, 2 * GRP], bf16,
                    name=f"cc_in{u}_{hb}", tag=f"cc_in{u}_{hb}")
                cc_out[(u, hb)] = dram.tile(
                    [N_CORES, P, 2 * GRP], bf16,
                    name=f"cc_out{u}_{hb}", tag=f"cc_out{u}_{hb}")
        TG = 2 * GRP  # 128-token slot per dst core per half-batch

        # persistent per-batch tiles (2 rotating buffers each)
        qt_b = [qkv_pool.tile([P, SEQ], bf16, tag="qt", name="qt")
                for _ in range(2)]
        kt_b = [qkv_pool.tile([P, SEQ], bf16, tag="kt", name="kt")
                for _ in range(2)]
        # V natural tiles padded to 193 wide so both heads' O-matmul
        # weights can be read as full 128-column tiles (enables FWL);
        # rows 65+ of the accumulators are junk and never read.
        vn_b = [v_pool.tile([P, N_JT_B, 193], bf16, tag="v", name="v")
                for _ in range(2)]
        # ones columns + FWL padding of the V tiles are written once
        for vb in vn_b:
            for jt in range(N_JT_B):
                nc.vector.tensor_copy(vb[:, jt, 64:65], onecol_f32[:])
                nc.vector.tensor_copy(vb[:, jt, 129:130], onecol_f32[:])
                nc.vector.memset(vb[:, jt, 130:193], 0.0)

        qt = lambda u: qt_b[u % 2]
        kt = lambda u: kt_b[u % 2]
        vn = lambda u: vn_b[u % 2]

        xt_tiles = {}

        def emit_xt_dma(u, icu):
            t0 = u * SEQ + icu * FT
            xs = []
            for k in range(KO):
                x = xt_pool.tile([P, FT], bf16, tag="xt", name="xt")
                nc.sync.dma_start(x[:], xT[ts(k, P), ds(t0, FT)])
                xs.append(x)
            xt_tiles[(u, icu)] = xs

        def mm_piece(u, icu, which):
            """One projection (q/k/v) of one 512-token chunk: 8 matmuls
            + DVE bias evacuation (+ V transposes)."""
            xs = xt_tiles[(u, icu)]
            cs = ds(icu * FT, FT)
            w_sb, b_sb = ((wq_sb, bq_sb), (wk_sb, bk_sb),
                          (wv_sb, bv_sb))[which]
            ps = ps_a.tile([P, FT], f32, tag="psa", name="ps")
            for k in range(KO):
                nc.tensor.matmul(ps[:], w_sb[:, k], xs[k][:],
                                 start=(k == 0), stop=(k == KO - 1))
            if which == 0:
                nc.vector.tensor_scalar_add(qt(u)[:, cs], ps[:], b_sb[:])
            elif which == 1:
                nc.vector.tensor_scalar_add(kt(u)[:, cs], ps[:], b_sb[:])
            else:
                vt_tmp = vt_pool.tile([P, FT], bf16, tag="vt",
                                      name="vt_tmp")
                nc.vector.tensor_scalar_add(vt_tmp[:], ps[:], b_sb[:])
                for j in range(FT // P):
                    jt = icu * (FT // P) + j
                    vtr = ps_a.tile([P, P], bf16, tag="psa", name="vtr")
                    nc.tensor.transpose(vtr[:], vt_tmp[:, ts(j, P)],
                                        ident[:])
                    nc.vector.tensor_copy(vn(u)[:, jt, 0:D_HEAD],
                                          vtr[:, 0:D_HEAD])
                    nc.vector.tensor_copy(vn(u)[:, jt, 65:129],
                                          vtr[:, D_HEAD:P])

        # ---- normalize + A2A ----
        norm_state = {}

        def norm_start(u, icc, po_sb):
            """DVE half of the normalize: reciprocal of the denominators
            (row 64 of the accumulators)."""
            recbs = []
            for h in range(2):
                row = ev_pool.tile([1, FT], f32, tag="row", name="row")
                nc.vector.tensor_copy(row[:], po_sb[h][64:65, :])
                rec = ev_pool.tile([1, FT], f32, tag="rec", name="rec")
                nc.vector.reciprocal_approx_fast(rec[:], row[:])
                recb = ev_pool.tile([1, FT], bf16, tag="recb",
                                    name="recb")
                nc.vector.tensor_copy(recb[:], rec[:])
                recbs.append(recb)
            norm_state[(u, icc)] = (po_sb, recbs)

        def norm_finish(u, icc):
            """PE broadcast + DVE scale + A2A input DMA; fires the
            half-batch collective after the odd chunk."""
            po_sb, recbs = norm_state.pop((u, icc))
            cin = cc_in[(u, icc // 2)]
            g = icc % 2
            for h in range(2):
                ps_r = ps_a.tile([P, FT], f32, tag="psa", name="ps_r")
                nc.tensor.matmul(ps_r[:], ones_bf[:], recbs[h][:],
                                 start=True, stop=True)
                av = av_pool.tile([D_HEAD, FT], bf16, tag="av",
                                  name="av")
                nc.vector.tensor_tensor(av[:], po_sb[h][0:D_HEAD, :],
                                        ps_r[0:D_HEAD, :],
                                        mybir.AluOpType.mult)
                for m in range(4):
                    nc.gpsimd.dma_start(
                        cin[4 * g + m, ds(h * D_HEAD, D_HEAD), :],
                        av[:, ds(m * TG, TG)])
            if g == 1:
                nc.gpsimd.collective_compute(
                    "AllToAll", mybir.AluOpType.bypass,
                    replica_groups=[list(range(N_CORES))],
                    ins=[cin.opt()], outs=[cc_out[(u, icc // 2)].opt()])

        # ---- out-proj pieces ----
        avt_tiles = {}

        def avt_dma(u, tl):
            """Prefetch the re-sharded attn_v^T for token tile tl."""
            av_t = avt_pool.tile([P, KO, 2 * GRP], bf16, tag="avt2",
                                 name="av_t")
            nc.sync.dma_start(
                av_t[:],
                cc_out[(u, tl)][:].rearrange("s p r -> p s r"))
            avt_tiles[(u, tl)] = av_t

        def proj_piece(u, tl, fc):
            av_t = avt_tiles[(u, tl)]
            ps_d = ps_a.tile([P, FT], f32, tag="psa", name="ps_d")
            for k in range(KO):
                nc.tensor.matmul(ps_d[:], av_t[:, k],
                                 wout_sb[:, k, ds(fc * FT, FT)],
                                 start=(k == 0), stop=(k == KO - 1))
            o_sb = out_pool.tile([P, FT], f32, tag="osb",
                                 name="o_sb")
            nc.vector.tensor_tensor(o_sb[:], ps_d[:],
                                    bout_sb[:, ds(fc * FT, FT)],
                                    mybir.AluOpType.add)
            nc.gpsimd.dma_start(
                out[ds(u * TBLK + tl * P, P), ds(fc * FT, FT)],
                o_sb[:])

        # ---- attention i-chunk with interleaved fillers ----
        N_PRE = 2
        pre_state = {"pre": []}

        def emit_s_exp(uu, ii, j):
            """S matmuls for key tile j of slot (uu, ii) + the exp."""
            qs = ds(ii * FT, FT)
            ps_sc = ps_s.tile([P, 2, FT], f32, tag="pss", name="ps_sc")
            for h in range(2):
                hp = ds(h * D_HEAD, D_HEAD)
                nc.tensor.matmul(ps_sc[:, h, :], kt(uu)[hp, ts(j, P)],
                                 qt(uu)[hp, qs], start=True, stop=True)
            pexp = p_pool.tile([P, 2, FT], bf16, tag="pexp",
                               name="pexp")
            nc.scalar.activation(pexp[:], ps_sc[:], AF.Exp,
                                 scale=float(SCALE))
            return pexp

        def attention_slot(u, icc, fillers, next_ctx):
            fq = deque(fillers)
            po = [ps_o.tile([P, FT], f32, tag="pso", name="pso")
                  for _ in range(2)]
            pre = pre_state["pre"]
            for j in range(N_JT_B):
                pexp = pre[j] if j < len(pre) else emit_s_exp(u, icc, j)
                st, sp = (j == 0), (j == N_JT_B - 1)
                for h in range(2):
                    nc.tensor.matmul(po[h][:],
                                     vn(u)[:, j, ds(h * 65, P)],
                                     pexp[:, h, :], start=st, stop=sp)
            # pre-emit the next slot's first S/exp pairs so ScalarE has
            # work during the filler block and the next j-loop starts
            # with its O matmuls unblocked
            pre_state["pre"] = (
                [emit_s_exp(*next_ctx, j2) for j2 in range(N_PRE)]
                if next_ctx is not None else [])
            while fq:
                fq.popleft()()
            po_sb = [None, None]
            for h in range(2):
                po_sb[h] = posb_pool.tile([P, FT], f32, tag="posb",
                                          name="po_sb")
                nc.vector.tensor_copy(po_sb[h][0:65, :], po[h][0:65, :])
            return po_sb

        # ---- prologue: batch-0 QKV, deferred big consts ----
        emit_xt_dma(0, 0)
        emit_xt_dma(0, 1)
        nc.gpsimd.dma_start(wout_sb[:],
                            w_out.rearrange("(ko p) f -> p ko f", p=P))
        nc.gpsimd.dma_start(bout_sb[:], b_out[:])
        for which in range(3):
            mm_piece(0, 0, which)
        emit_xt_dma(0, 2)
        for which in range(3):
            mm_piece(0, 1, which)
        emit_xt_dma(0, 3)
        for which in range(3):
            mm_piece(0, 2, which)
        for which in range(3):
            mm_piece(0, 3, which)

        # ---- main software-pipelined slot loop ----
        prev = None          # (u, icc) whose normalize is pending
        po_prev = None
        for u in range(BATCH):
            for icc in range(N_CHUNK_B):
                if u + 1 < BATCH:
                    emit_xt_dma(u + 1, icc)
                if prev is not None:
                    norm_start(*prev, po_prev)
                fillers = []
                if prev is not None:
                    pu, pic = prev
                    fillers.append(lambda pu=pu, pic=pic:
                                   norm_finish(pu, pic))
                if u + 1 < BATCH:
                    for which in range(3):
                        fillers.append(
                            lambda w=which, uu=u + 1, ii=icc:
                            mm_piece(uu, ii, w))
                # out-proj / avt prefetch schedule
                if icc == 0 and u >= 1:
                    fillers.append(lambda uu=u - 1: proj_piece(uu, 0, 0))
                    fillers.append(lambda uu=u - 1: proj_piece(uu, 0, 1))
                if icc == 1 and u >= 1:
                    fillers.append(lambda uu=u - 1: avt_dma(uu, 1))
                if icc == 2 and u >= 1:
                    fillers.append(lambda uu=u - 1: proj_piece(uu, 1, 0))
                    fillers.append(lambda uu=u - 1: proj_piece(uu, 1, 1))
                if icc == 3:
                    fillers.append(lambda uu=u: avt_dma(uu, 0))

                nxt = (u, icc + 1) if icc + 1 < N_CHUNK_B else (
                    (u + 1, 0) if u + 1 < BATCH else None)
                po_sb = attention_slot(u, icc, fillers, nxt)
                prev = (u, icc)
                po_prev = po_sb

        # ---- tail ----
        norm_start(*prev, po_prev)
        norm_finish(*prev)
        proj_piece(BATCH - 1, 0, 0)
        proj_piece(BATCH - 1, 0, 1)
        avt_dma(BATCH - 1, 1)
        proj_piece(BATCH - 1, 1, 0)
        proj_piece(BATCH - 1, 1, 1)


def _prep_inputs(x, w_qkv, b_qkv, w_out, b_out):
    x = np.asarray(x, dtype=np.float32)
    w_qkv = np.asarray(w_qkv, dtype=np.float32)
    b_qkv = np.asarray(b_qkv, dtype=np.float32)
    w_out = np.asarray(w_out, dtype=np.float32)
    b_out = np.asarray(b_out, dtype=np.float32)
    bf = ml_dtypes.bfloat16

    xT = np.ascontiguousarray(x.reshape(T_FULL, D_MODEL).T).astype(bf)
    w_out_bf = w_out.astype(bf)
    bout_bc = np.ascontiguousarray(np.broadcast_to(b_out, (P, D_MODEL)))

    in_maps = []
    for c in range(N_CORES):
        cols = slice(c * P, (c + 1) * P)
        in_maps.append({
            "xT": xT,
            "wq": np.ascontiguousarray(w_qkv[:, 0:D_MODEL][:, cols]).astype(bf),
            "wk": np.ascontiguousarray(
                w_qkv[:, D_MODEL:2 * D_MODEL][:, cols]).astype(bf),
            "wv": np.ascontiguousarray(
                w_qkv[:, 2 * D_MODEL:3 * D_MODEL][:, cols]).astype(bf),
            "bqkv": np.ascontiguousarray(np.stack([
                b_qkv[0:D_MODEL][cols],
                b_qkv[D_MODEL:2 * D_MODEL][cols],
                b_qkv[2 * D_MODEL:3 * D_MODEL][cols]])),
            "w_out": w_out_bf,
            "b_out": bout_bc,
        })
    return in_maps


def kernel(x, w_qkv, b_qkv, w_out, b_out, _trace=False):
    if "nc" not in _CACHE:
        _CACHE["nc"] = build()
    nc = _CACHE["nc"]
    in_maps = _prep_inputs(x, w_qkv, b_qkv, w_out, b_out)
    res = bass_utils.run_bass_kernel_spmd(
        nc, in_maps, core_ids=list(range(N_CORES)), trace=_trace)
    _CACHE["last_result"] = res
    # reorder interleaved 128-token groups:
    # core c's out row (u*256 + hb*128 + r) is global token
    # u*2048 + hb*1024 + c*128 + r
    full = np.empty((T_FULL, D_MODEL), np.float32)
    for c in range(N_CORES):
        r = res.results[c]["out"]
        for u in range(BATCH):
            for hb in range(2):
                src = u * TBLK + hb * 128
                dst = u * SEQ + hb * 1024 + c * 128
                full[dst:dst + 128] = r[src:src + 128]
    return full.reshape(BATCH, SEQ, D_MODEL)


if __name__ == "__main__":
    rng = np.random.default_rng(0)
    x = rng.standard_normal((BATCH, SEQ, D_MODEL), dtype=np.float32)
    w_qkv = rng.standard_normal((D_MODEL, 3 * D_MODEL),
                                dtype=np.float32) / 32.0
    b_qkv = rng.standard_normal((3 * D_MODEL,), dtype=np.float32) * 0.02
    w_out = rng.standard_normal((D_MODEL, D_MODEL), dtype=np.float32) / 32.0
    b_out = rng.standard_normal((D_MODEL,), dtype=np.float32) * 0.02
    got = kernel(x, w_qkv, b_qkv, w_out, b_out)
    print("out shape:", got.shape)
